# revision 1
# baseline (speedup 1.0000x reference)
# Trainium2 Bass kernel for nn_CvtLstm: ConvLSTM cell with 4-branch,
# 4-head spatial attention. Data-parallel over batch N=32 across 8
# NeuronCores (4 samples per core); weights replicated to every core.
#
# Per-core layout: channels on partitions, flattened 16x16 spatial (256)
# on the free dim. conv3x3 = 9 shifted matmuls reading a zero-padded
# [128, 2, 18, 18] tile. Attention scores are computed directly in the
# transposed [d, q] layout (lhsT = per-head k rows, K=32 row-partial
# matmuls); exp on the ACT engine with no max subtraction (scores lie in
# [-9, 8]); the PV product and the softmax denominator Z come from one
# M=64 matmul per (head, d-chunk) whose weight columns are [vT_g | ones];
# normalization is a DMA head-restack + reciprocal + multiply.
#
# Hardware constraint honored throughout: two row-partial matmuls at
# different row groups back-to-back fault the device (LDWEIGHTS pull-ahead
# across non-conflicting row groups). A full-row K>=64... strictly a
# row-range-conflicting matmul between them is safe; emission order is the
# per-engine execution order, so score matmuls are ordered g-outer/c-inner
# with full-row dummy separators at group changes, and iterations are
# separated by the (full-row) PV/Z matmuls of the previous iteration.

import numpy as np

N, I, H, W = 32, 64, 16, 16
R, CM, A, HEADS, HC = 128, 128, 128, 4, 32
HW = H * W           # 256
S = 4                # samples per core
NCORES = 8

_CACHE = {}


def _build_program():
    import contextlib
    import concourse.bacc as bacc
    import concourse.mybir as mybir
    import concourse.tile as tile
    import concourse.bass as bass

    F32 = mybir.dt.float32
    F32R = mybir.dt.float32r
    AF = mybir.ActivationFunctionType

    nc = bacc.Bacc("TRN2", target_bir_lowering=False, debug=False)

    def dram(name, shape, kind="ExternalInput"):
        return nc.dram_tensor(name, list(shape), F32, kind=kind).ap()

    xin = dram("xin", [S, I, HW])
    hin = dram("hin", [S, R, HW])
    cin = dram("cin", [S, R, HW])
    zpad = dram("zpad", [128, 648])
    winTd = dram("winT", [I, R])
    b_ind = dram("b_in", [R, 1])
    wconvTd = dram("wconvT", [128, 2, 9, 128])
    wqkTd = dram("wqkT", [128, 2, 4, 128])
    wvTd = dram("wvT", [128, 2, 256])
    onesd = dram("onesd", [128, 32])
    wtokTd = dram("wtokT", [128, 4, 4, 128])
    btokd = dram("btok", [128, 4])
    wskipTd = dram("wskipT", [128, 4, 2, 128])
    woutTd = dram("woutT", [128, 128])
    boutd = dram("bout", [128, 1])
    yout = dram("yout", [S, R, HW], kind="ExternalOutput")

    QSRC = [0, 0, 1, 1]   # q source per branch: 0=xc, 1=hc
    KSRC = [0, 1, 0, 1]   # k/v source per branch
    BORDER = [3, 1, 2, 0]  # per-pass branch order (b3 = pure hc, earliest)

    with tile.TileContext(nc) as tc:
        with contextlib.ExitStack() as ctx:
            wpool = ctx.enter_context(tc.tile_pool(name="wts", bufs=1))
            sbA = ctx.enter_context(tc.tile_pool(name="sbA", bufs=2))
            sbB = ctx.enter_context(tc.tile_pool(name="sbB", bufs=2))
            stp = ctx.enter_context(tc.tile_pool(name="st", bufs=2, space="PSUM"))
            azp = ctx.enter_context(tc.tile_pool(name="az", bufs=1, space="PSUM"))
            pwp = ctx.enter_context(tc.tile_pool(name="pw", bufs=2, space="PSUM"))

            # ---------------- weights to SBUF ----------------
            def wload(name, src, shape, dt=F32R):
                t = wpool.tile(shape, dt, tag=name, name=name)
                nc.sync.dma_start(out=t, in_=src.bitcast(dt) if dt == F32R else src)
                return t

            winT_s = wload("winT", winTd, [I, R])
            wconvT_s = wload("wconvT", wconvTd, [128, 2, 9, 128])
            wqkT_s = wload("wqkT", wqkTd, [128, 2, 4, 128])
            wvT_s = wload("wvT", wvTd, [128, 2, 256])
            ones_s = wload("ones", onesd, [128, 32])
            wtokT_s = wload("wtokT", wtokTd, [128, 4, 4, 128])
            wskipT_s = wload("wskipT", wskipTd, [128, 4, 2, 128])
            woutT_s = wload("woutT", woutTd, [128, 128])
            b_in_s = wload("b_in", b_ind, [R, 1], F32)
            btok_s = wload("btok", btokd, [128, 4], F32)
            bout_s = wload("bout", boutd, [128, 1], F32)

            # ---------------- per-pass state ----------------
            xc_sb = [None, None]
            hc_sb = [None, None]
            q_sb = [[None] * 4, [None] * 4]
            k_sb = [[None] * 4, [None] * 4]
            vt_sb = [[None] * 4, [None] * 4]
            a_all = [None, None]
            cprev_sb = [None, None]
            gate_sb = [[None] * 4, [None] * 4]

            def emit_input_pads(p):
                """zero-padded xt/h tiles + x2 + XT matmul + tanh."""
                xt_pad = sbA.tile([128, 648], F32R, tag="xtpad", name="xtpad")
                h_pad = sbA.tile([128, 648], F32R, tag="hpad", name="hpad")
                x2 = sbA.tile([64, 2, 256], F32R, tag="x2", name="x2")
                nc.sync.dma_start(out=xt_pad, in_=zpad.bitcast(F32R))
                nc.sync.dma_start(out=h_pad, in_=zpad.bitcast(F32R))
                hv = h_pad.rearrange("p (s y x) -> p s y x", s=2, y=18, x=18)
                for s in range(2):
                    nc.sync.dma_start(
                        out=hv[:, s, 1:17, 1:17],
                        in_=hin[2 * p + s].rearrange(
                            "c (h w) -> c h w", h=16).bitcast(F32R))
                nc.sync.dma_start(
                    out=x2,
                    in_=xin[2 * p:2 * p + 2].rearrange("s c q -> c s q").bitcast(F32R))
                XT = pwp.tile([128, 512], F32, tag="pw", name="XT")
                nc.tensor.matmul(out=XT, lhsT=winT_s,
                                 rhs=x2.rearrange("p s q -> p (s q)"),
                                 start=True, stop=True)
                xv = xt_pad.rearrange("p (s y x) -> p s y x", s=2, y=18, x=18)
                nc.scalar.activation(
                    out=xv[:, :, 1:17, 1:17],
                    in_=XT.rearrange("p (s h w) -> p s h w", s=2, h=16, w=16),
                    func=AF.Tanh, bias=b_in_s)
                return xt_pad, h_pad

            def emit_conv(p, src, pad):
                """3x3 SAME conv via 9 shifted matmuls; src 0=xc, 1=hc."""
                CP = pwp.tile([128, 512], F32, tag="pw", name="CP")
                pv = pad.rearrange("p (s y x) -> p s y x", s=2, y=18, x=18)
                for t in range(9):
                    ky, kx = divmod(t, 3)
                    nc.tensor.matmul(out=CP, lhsT=wconvT_s[:, src, t, :],
                                     rhs=pv[:, :, ky:ky + 16, kx:kx + 16],
                                     start=(t == 0), stop=(t == 8))
                dst = sbA.tile([128, 512], F32R, tag=("xc" if src == 0 else "hc"), name=("xc" if src == 0 else "hc"))
                nc.vector.tensor_copy(dst, CP)
                if src == 0:
                    xc_sb[p] = dst
                else:
                    hc_sb[p] = dst

            def emit_qk(p, b):
                srcq = xc_sb[p] if QSRC[b] == 0 else hc_sb[p]
                srck = xc_sb[p] if KSRC[b] == 0 else hc_sb[p]
                QB = pwp.tile([128, 512], F32, tag="pw", name="QB")
                nc.tensor.matmul(out=QB, lhsT=wqkT_s[:, 0, b, :], rhs=srcq,
                                 start=True, stop=True)
                q_sb[p][b] = sbB.tile([128, 512], F32R, tag=f"q{b}", name=f"q{b}")
                nc.vector.tensor_copy(q_sb[p][b], QB)
                KB = pwp.tile([128, 512], F32, tag="pw", name="KB")
                nc.tensor.matmul(out=KB, lhsT=wqkT_s[:, 1, b, :], rhs=srck,
                                 start=True, stop=True)
                k_sb[p][b] = sbB.tile([128, 512], F32R, tag=f"k{b}", name=f"k{b}")
                nc.vector.tensor_copy(k_sb[p][b], KB)

            def emit_vt(p, src):
                """vT for the two branches fed by `src`; fills [vT_g | ones]
                64-wide head blocks of vt_sb[b] = [128, (s c) 4, 256]."""
                b0 = src            # branches (0,2) from xc, (1,3) from hc
                for b in (b0, b0 + 2):
                    if vt_sb[p][b] is None:
                        vt_sb[p][b] = sbB.tile([128, 1024], F32R, tag=f"vt{b}", name=f"vt{b}")
                src_sb = xc_sb[p] if src == 0 else hc_sb[p]
                sv = src_sb.rearrange("p (s c d) -> p s c d", s=2, c=2)
                for s in range(2):
                    for c in range(2):
                        VT = pwp.tile([128, 256], F32, tag="pw", name="VT")
                        nc.tensor.matmul(out=VT, lhsT=sv[:, s, c, :],
                                         rhs=wvT_s[:, src, :],
                                         start=True, stop=True)
                        sc = s * 2 + c
                        for j, b in enumerate((b0, b0 + 2)):
                            dst = vt_sb[p][b][:, sc * 256:(sc + 1) * 256]
                            dst = dst.rearrange("p (g d) -> p g d", g=4)[:, :, 0:32]
                            srcv = VT[:, j * 128:(j + 1) * 128].rearrange(
                                "p (g d) -> p g d", g=4)
                            nc.vector.tensor_copy(dst, srcv)
                for b in (b0, b0 + 2):
                    dst = vt_sb[p][b].rearrange(
                        "p (n d) -> p n d", n=16)[:, :, 32:64]
                    srcap = bass.AP(tensor=onesd.tensor, offset=0,
                                    ap=[[32, 128], [0, 16], [1, 32]])
                    nc.sync.dma_start(out=dst, in_=srcap.bitcast(F32R))

            def emit_cprev(p):
                cprev_sb[p] = sbA.tile([128, 512], F32, tag="cprev", name="cprev")
                nc.sync.dma_start(
                    out=cprev_sb[p],
                    in_=cin[2 * p:2 * p + 2].rearrange("s c q -> c s q"))

            # ---------------- attention iteration pieces ----------------
            def emit_scores_exp(p, b, s):
                """returns pT tile [128, 2048] f32r = exp(scores^T), layout
                (g, c, q) 4x2x256."""
                kv = k_sb[p][b].rearrange("p (s c d) -> p s c d", s=2, c=2)
                qv = q_sb[p][b].rearrange("p (s q) -> p s q", s=2)
                pT = sbB.tile([128, 2048], F32R, tag="pt", name="pT")
                for h in range(2):
                    ST = stp.tile([128, 1024], F32, tag="st", name="ST")
                    for gg in range(2):
                        g = 2 * h + gg
                        if (h, gg) != (0, 0):
                            # full-row dummy separator into a slice the next
                            # score matmul overwrites (start=True clears it)
                            dsl = ST[0:32, 512:544] if gg == 1 else ST[0:32, 0:32]
                            nc.tensor.matmul(out=dsl, lhsT=ones_s,
                                             rhs=ones_s, start=True, stop=True,
                                             skip_group_check=True)
                        for c in range(2):
                            nc.tensor.matmul(
                                out=ST[:, gg * 512 + c * 256:gg * 512 + c * 256 + 256],
                                lhsT=kv[32 * g:32 * g + 32, s, c, :],
                                rhs=qv[32 * g:32 * g + 32, s, :],
                                start=True, stop=True, skip_group_check=True,
                                tile_position=(32 * g, 0))
                    nc.scalar.activation(out=pT[:, h * 1024:(h + 1) * 1024],
                                         in_=ST, func=AF.Exp)
                return pT

            def emit_pvz(p, b, s, pT):
                AZ = azp.tile([64, 1024], F32, tag="az", name="AZ")
                for g in range(4):
                    for c in range(2):
                        sc = s * 2 + c
                        nc.tensor.matmul(
                            out=AZ[0:64, g * 256:(g + 1) * 256],
                            lhsT=vt_sb[p][b][:, sc * 256 + 64 * g:sc * 256 + 64 * g + 64],
                            rhs=pT[:, g * 512 + c * 256:g * 512 + c * 256 + 256],
                            start=(c == 0), stop=(c == 1), skip_group_check=True)
                return AZ

            def emit_norm(p, b, s, AZ):
                a_flat = sbB.tile([64, 1024], F32, tag="aflat", name="a_flat")
                nc.vector.tensor_copy(a_flat, AZ)
                a_tmp = sbB.tile([128, 256], F32R, tag="atmp", name="a_tmp")
                zb = sbB.tile([128, 256], F32, tag="zb", name="zb")
                rz = sbB.tile([128, 256], F32, tag="rz", name="rz")
                for g in range(4):
                    nc.sync.dma_start(
                        out=a_tmp[32 * g:32 * g + 32, :],
                        in_=a_flat[0:32, g * 256:(g + 1) * 256].bitcast(F32R))
                    nc.sync.dma_start(
                        out=zb[32 * g:32 * g + 32, :],
                        in_=a_flat[32:64, g * 256:(g + 1) * 256])
                nc.vector.reciprocal_approx_fast(out=rz, in_=zb)
                if a_all[p] is None:
                    a_all[p] = sbA.tile([128, 2048], F32R, tag="aall", name="a_all")
                slot = b * 2 + s
                nc.vector.tensor_mul(a_all[p][:, slot * 256:(slot + 1) * 256],
                                     a_tmp.bitcast(F32), rz)

            # ---------------- gates / state / output ----------------
            def emit_gate(p, gi):
                G = pwp.tile([128, 512], F32, tag="pw", name="G")
                av = a_all[p].rearrange("p (b s q) -> p b (s q)", b=4, s=2)
                for b in range(4):
                    nc.tensor.matmul(out=G, lhsT=wtokT_s[:, gi, b, :],
                                     rhs=av[:, b, :],
                                     start=(b == 0), stop=False)
                nc.tensor.matmul(out=G, lhsT=wskipT_s[:, gi, 0, :],
                                 rhs=xc_sb[p], start=False, stop=False)
                nc.tensor.matmul(out=G, lhsT=wskipT_s[:, gi, 1, :],
                                 rhs=hc_sb[p], start=False, stop=True)
                gate_sb[p][gi] = sbA.tile([128, 512], F32, tag=f"gate{gi}", name=f"gate{gi}")
                func = AF.Tanh if gi == 2 else AF.Sigmoid
                nc.scalar.activation(out=gate_sb[p][gi], in_=G, func=func,
                                     bias=btok_s[:, gi:gi + 1])

            def emit_update_out(p):
                gi_, gf_, gg_, go_ = gate_sb[p]
                fc = sbA.tile([128, 512], F32, tag="fc", name="fc")
                nc.vector.tensor_mul(fc, gf_, cprev_sb[p])
                ig = sbA.tile([128, 512], F32, tag="ig", name="ig")
                nc.vector.tensor_mul(ig, gi_, gg_)
                cs = sbA.tile([128, 512], F32, tag="c", name="cs")
                nc.vector.tensor_add(cs, fc, ig)
                tcs = sbA.tile([128, 512], F32, tag="tc", name="tcs")
                nc.scalar.activation(out=tcs, in_=cs, func=AF.Tanh)
                hs = sbA.tile([128, 512], F32R, tag="h", name="hs")
                nc.vector.tensor_mul(hs, go_, tcs)
                OUT = pwp.tile([128, 512], F32, tag="pw", name="OUT")
                nc.tensor.matmul(out=OUT, lhsT=woutT_s, rhs=hs,
                                 start=True, stop=True)
                osb = sbA.tile([128, 512], F32, tag="out", name="osb")
                nc.vector.tensor_scalar_add(osb, OUT, bout_s[:, 0:1])
                nc.sync.dma_start(
                    out=yout[2 * p:2 * p + 2].rearrange("s c q -> c s q"),
                    in_=osb.rearrange("p (s q) -> p s q", s=2))

            # ---------------- emission schedule ----------------
            # prologue: pass-0 essentials up to branch 3 (pure hc)
            xt_pad0, h_pad0 = emit_input_pads(0)
            emit_conv(0, 1, h_pad0)      # hc pass0
            emit_qk(0, 3)
            emit_vt(0, 1)                # vT for b1, b3 (hc source)
            pads1 = [None]

            def filler(i):
                if i == 0:
                    emit_conv(0, 0, xt_pad0)          # xc pass0
                elif i == 1:
                    emit_qk(0, 1)
                    emit_qk(0, 2)
                elif i == 2:
                    emit_qk(0, 0)
                    emit_vt(0, 0)
                    emit_cprev(0)
                elif i == 3:
                    pads1[0] = emit_input_pads(1)
                elif i == 4:
                    emit_conv(1, 1, pads1[0][1])      # hc pass1
                elif i == 5:
                    emit_conv(1, 0, pads1[0][0])      # xc pass1
                elif i == 6:
                    emit_qk(1, 3)
                    emit_vt(1, 1)
                elif i == 7:
                    emit_qk(1, 1)
                    emit_qk(1, 2)
                elif i == 8:
                    emit_qk(1, 0)
                    emit_vt(1, 0)
                    emit_cprev(1)
                elif i in (9, 10, 11, 12):
                    emit_gate(0, i - 9)
                elif i == 13:
                    emit_update_out(0)

            iters = [(p, b, s) for p in (0, 1) for b in BORDER for s in (0, 1)]
            prev = None
            for i, (p, b, s) in enumerate(iters):
                pT = emit_scores_exp(p, b, s)
                if prev is not None:
                    pp, pb, ps, ppT = prev
                    AZ = emit_pvz(pp, pb, ps, ppT)
                    emit_norm(pp, pb, ps, AZ)
                prev = (p, b, s, pT)
                filler(i)
            pp, pb, ps, ppT = prev
            AZ = emit_pvz(pp, pb, ps, ppT)
            emit_norm(pp, pb, ps, AZ)
            for gi in range(4):
                emit_gate(1, gi)
            emit_update_out(1)

    nc.compile()
    return nc


def _prep_shared(inputs):
    f = np.float32
    c = np.ascontiguousarray
    W_cx, W_ch = np.asarray(inputs["W_cx"], f), np.asarray(inputs["W_ch"], f)
    W_q, W_k, W_v = (np.asarray(inputs[k], f) for k in ("W_q", "W_k", "W_v"))
    W_tok, W_skip = np.asarray(inputs["W_tok"], f), np.asarray(inputs["W_skip"], f)
    shared = {
        "zpad": np.zeros((128, 648), f),
        "winT": c(np.asarray(inputs["W_in"], f).T),
        "b_in": c(np.asarray(inputs["b_in"], f).reshape(R, 1)),
        # [i, src, tap, o]
        "wconvT": c(np.stack([W_cx.transpose(1, 2, 3, 0).reshape(128, 9, 128),
                              W_ch.transpose(1, 2, 3, 0).reshape(128, 9, 128)],
                             axis=1)),
        # [c, (q|k), b, a]
        "wqkT": c(np.stack([W_q.transpose(2, 0, 1), W_k.transpose(2, 0, 1)],
                           axis=1)),
        # [c, srcpair, a-pair]: xc feeds branches (0,2), hc feeds (1,3)
        "wvT": c(np.stack([
            np.concatenate([W_v[0].T, W_v[2].T], axis=1),
            np.concatenate([W_v[1].T, W_v[3].T], axis=1)], axis=1)),
        "onesd": np.ones((128, 32), f),
        # [a, gate, branch, r]
        "wtokT": c(W_tok.transpose(3, 0, 1, 2)),
        "btok": c(np.asarray(inputs["b_tok"], f).T),
        # [c, gate, src, r]
        "wskipT": c(W_skip.transpose(3, 0, 1, 2)),
        "woutT": c(np.asarray(inputs["W_out"], f).T),
        "bout": c(np.asarray(inputs["b_out"], f).reshape(R, 1)),
    }
    return shared


def kernel(**inputs):
    from concourse.bass_utils import run_bass_kernel_spmd
    if "nc" not in _CACHE:
        _CACHE["nc"] = _build_program()
    nc = _CACHE["nc"]
    f = np.float32
    x = np.asarray(inputs["x"], f).reshape(N, I, HW)
    hp = np.asarray(inputs["h_prev"], f).reshape(N, R, HW)
    cp = np.asarray(inputs["c_prev"], f).reshape(N, R, HW)
    shared = _prep_shared(inputs)
    in_maps = []
    for ci in range(NCORES):
        sl = slice(S * ci, S * ci + S)
        m = dict(shared)
        m["xin"] = np.ascontiguousarray(x[sl])
        m["hin"] = np.ascontiguousarray(hp[sl])
        m["cin"] = np.ascontiguousarray(cp[sl])
        in_maps.append(m)
    res = run_bass_kernel_spmd(nc, in_maps, core_ids=list(range(NCORES)))
    y = np.concatenate([r["yout"].reshape(S, R, H, W) for r in res.results],
                       axis=0)
    return y.astype(np.float32)



# revision 5
# speedup vs baseline: 1.4142x; 1.4142x over previous
# Trainium2 Bass kernel for nn_CvtLstm: ConvLSTM cell with 4-branch,
# 4-head spatial attention. Data-parallel over batch N=32 across 8
# NeuronCores (4 samples per core); weights replicated to every core.
#
# Per-core layout: channels on partitions, flattened 16x16 spatial (256)
# on the free dim. conv3x3 = 9 shifted matmuls reading a zero-padded
# [128, 2, 18, 18] tile (borders zeroed by Pool-engine memset, not DMA).
# Attention scores are computed directly in the transposed [d, q] layout
# (lhsT = per-head k rows, K=32 row-partial matmuls); exp on the ACT
# engine with no max subtraction (scores lie in [-9, 8]).
#
# Softmax normalization has no PSUM->SBUF copy and only 4 small DMAs per
# iteration: the PV product uses weight columns [vT_g | ones] (M=64) and
# writes a two-head-stacked PSUM tile AZ2 [128, 512] (head g at
# partitions 64*(g%2), columns 256*(g//2); its Z rows 32 below). The
# reciprocal reads AZ2 straight from PSUM, the normalizing multiply also
# reads PSUM (doubling as the copy-out), and the normalized [64, 512]
# tile is restacked into a_all with 4 per-head DMAs, 2 on the shared
# HWDGE and 2 on the Pool engine's software DGE to halve HWDGE pressure.
#
# Hardware constraint honored throughout: two row-partial matmuls at
# different row groups back-to-back fault the device (LDWEIGHTS pull-
# ahead across non-conflicting row groups); a full-row matmul between
# them is safe. Emission order is the per-engine execution order, so the
# K=128 PV matmuls of the previous iteration are interleaved between the
# score groups of the current one, acting as separators (no dummy
# matmuls needed except in the very first iteration).
#
# Tail: pass-1 gates are split into a partial (skips + branches 3,1,2,
# accumulated during the loop, copied to SBUF) and a 2-matmul tail
# (branch 0), so after the last attention iteration only a short chain
# remains.

import numpy as np

N, I, H, W = 32, 64, 16, 16
R, CM, A, HEADS, HC = 128, 128, 128, 4, 32
HW = H * W           # 256
S = 4                # samples per core
NCORES = 8

_CACHE = {}


def _build_program():
    import contextlib
    import concourse.bacc as bacc
    import concourse.mybir as mybir
    import concourse.tile as tile
    import concourse.bass as bass

    F32 = mybir.dt.float32
    F32R = mybir.dt.float32r
    AF = mybir.ActivationFunctionType

    nc = bacc.Bacc("TRN2", target_bir_lowering=False, debug=False)

    def dram(name, shape, kind="ExternalInput"):
        return nc.dram_tensor(name, list(shape), F32, kind=kind).ap()

    xin = dram("xin", [S, I, HW])
    hin = dram("hin", [S, R, HW])
    cin = dram("cin", [S, R, HW])
    winTd = dram("winT", [I, R])
    b_ind = dram("b_in", [R, 1])
    wconvTd = dram("wconvT", [128, 2, 9, 128])
    wqkTd = dram("wqkT", [128, 2, 4, 128])
    wvTd = dram("wvT", [128, 2, 256])
    onesd = dram("onesd", [128, 32])
    wtokTd = dram("wtokT", [128, 4, 4, 128])
    btokd = dram("btok", [128, 4])
    wskipTd = dram("wskipT", [128, 4, 2, 128])
    woutTd = dram("woutT", [128, 128])
    boutd = dram("bout", [128, 1])
    yout = dram("yout", [S, R, HW], kind="ExternalOutput")

    QSRC = [0, 0, 1, 1]   # q source per branch: 0=xc, 1=hc
    KSRC = [0, 1, 0, 1]   # k/v source per branch
    BORDER = [3, 1, 2, 0]  # per-pass branch order (b3 = pure hc, earliest)

    with tile.TileContext(nc) as tc:
        with contextlib.ExitStack() as ctx:
            wpool = ctx.enter_context(tc.tile_pool(name="wts", bufs=1))
            sbA = ctx.enter_context(tc.tile_pool(name="sbA", bufs=2))
            sbB = ctx.enter_context(tc.tile_pool(name="sbB", bufs=2))
            gpp = ctx.enter_context(tc.tile_pool(name="gpp", bufs=1))
            stp = ctx.enter_context(tc.tile_pool(name="st", bufs=2, space="PSUM"))
            azp = ctx.enter_context(tc.tile_pool(name="az", bufs=2, space="PSUM"))
            pwp = ctx.enter_context(tc.tile_pool(name="pw", bufs=2, space="PSUM"))

            # ---------------- weights to SBUF ----------------
            # Loads are issued lazily in dependency-criticality order by the
            # emission schedule below (hc-conv + qk weights first, gate
            # weights last) so the first attention iteration starts early.
            def wload(name, src, shape, dt=F32R):
                t = wpool.tile(shape, dt, tag=name, name=name)
                nc.sync.dma_start(out=t, in_=src.bitcast(dt) if dt == F32R else src)
                return t

            # ---------------- per-pass state ----------------
            xc_sb = [None, None]
            hc_sb = [None, None]
            q_sb = [[None] * 4, [None] * 4]
            k_sb = [[None] * 4, [None] * 4]
            vt_sb = [[None] * 4, [None] * 4]
            a_all = [None, None]
            cprev_sb = [None, None]
            gate_sb = [[None] * 4, [None] * 4]
            gpart_sb = [None] * 4          # pass-1 gate partials

            W_ = {}

            def emit_input_pads(p):
                """zero-padded xt/h tiles + x2 + XT matmul + tanh.
                Borders zeroed by Pool memset (no DRAM traffic)."""
                xt_pad = sbA.tile([128, 648], F32R, tag="xtpad", name="xtpad")
                h_pad = sbA.tile([128, 648], F32R, tag="hpad", name="hpad")
                x2 = sbA.tile([64, 2, 256], F32R, tag="x2", name="x2")
                nc.gpsimd.memset(xt_pad, 0.0)
                nc.gpsimd.memset(h_pad, 0.0)
                hv = h_pad.rearrange("p (s y x) -> p s y x", s=2, y=18, x=18)
                for s in range(2):
                    nc.sync.dma_start(
                        out=hv[:, s, 1:17, 1:17],
                        in_=hin[2 * p + s].rearrange(
                            "c (h w) -> c h w", h=16).bitcast(F32R))
                nc.sync.dma_start(
                    out=x2,
                    in_=xin[2 * p:2 * p + 2].rearrange("s c q -> c s q").bitcast(F32R))
                XT = pwp.tile([128, 512], F32, tag="pw", name="XT")
                nc.tensor.matmul(out=XT, lhsT=W_["winT"],
                                 rhs=x2.rearrange("p s q -> p (s q)"),
                                 start=True, stop=True)
                xv = xt_pad.rearrange("p (s y x) -> p s y x", s=2, y=18, x=18)
                nc.scalar.activation(
                    out=xv[:, :, 1:17, 1:17],
                    in_=XT.rearrange("p (s h w) -> p s h w", s=2, h=16, w=16),
                    func=AF.Tanh, bias=W_["b_in"])
                return xt_pad, h_pad

            def emit_conv(p, src, pad):
                """3x3 SAME conv via 9 shifted matmuls; src 0=xc, 1=hc."""
                CP = pwp.tile([128, 512], F32, tag="pw", name="CP")
                pv = pad.rearrange("p (s y x) -> p s y x", s=2, y=18, x=18)
                for t in range(9):
                    ky, kx = divmod(t, 3)
                    nc.tensor.matmul(out=CP, lhsT=W_["wconvT"][:, src, t, :],
                                     rhs=pv[:, :, ky:ky + 16, kx:kx + 16],
                                     start=(t == 0), stop=(t == 8))
                dst = sbA.tile([128, 512], F32R, tag=("xc" if src == 0 else "hc"), name=("xc" if src == 0 else "hc"))
                nc.vector.tensor_copy(dst, CP)
                if src == 0:
                    xc_sb[p] = dst
                else:
                    hc_sb[p] = dst

            def emit_qk(p, b):
                srcq = xc_sb[p] if QSRC[b] == 0 else hc_sb[p]
                srck = xc_sb[p] if KSRC[b] == 0 else hc_sb[p]
                QB = pwp.tile([128, 512], F32, tag="pw", name="QB")
                nc.tensor.matmul(out=QB, lhsT=W_["wqkT"][:, 0, b, :], rhs=srcq,
                                 start=True, stop=True)
                q_sb[p][b] = sbB.tile([128, 512], F32R, tag=f"q{b}", name=f"q{b}")
                nc.vector.tensor_copy(q_sb[p][b], QB)
                KB = pwp.tile([128, 512], F32, tag="pw", name="KB")
                nc.tensor.matmul(out=KB, lhsT=W_["wqkT"][:, 1, b, :], rhs=srck,
                                 start=True, stop=True)
                k_sb[p][b] = sbB.tile([128, 512], F32R, tag=f"k{b}", name=f"k{b}")
                nc.vector.tensor_copy(k_sb[p][b], KB)

            def emit_vt(p, src):
                """vT for the two branches fed by `src`; fills [vT_g | ones]
                64-wide head blocks of vt_sb[b] = [128, (s c) 4, 256]."""
                b0 = src            # branches (0,2) from xc, (1,3) from hc
                for b in (b0, b0 + 2):
                    if vt_sb[p][b] is None:
                        vt_sb[p][b] = sbB.tile([128, 1024], F32R, tag=f"vt{b}", name=f"vt{b}")
                src_sb = xc_sb[p] if src == 0 else hc_sb[p]
                sv = src_sb.rearrange("p (s c d) -> p s c d", s=2, c=2)
                for s in range(2):
                    for c in range(2):
                        VT = pwp.tile([128, 256], F32, tag="pw", name="VT")
                        nc.tensor.matmul(out=VT, lhsT=sv[:, s, c, :],
                                         rhs=W_["wvT"][:, src, :],
                                         start=True, stop=True)
                        sc = s * 2 + c
                        for j, b in enumerate((b0, b0 + 2)):
                            dst = vt_sb[p][b][:, sc * 256:(sc + 1) * 256]
                            dst = dst.rearrange("p (g d) -> p g d", g=4)[:, :, 0:32]
                            srcv = VT[:, j * 128:(j + 1) * 128].rearrange(
                                "p (g d) -> p g d", g=4)
                            nc.vector.tensor_copy(dst, srcv)
                for b in (b0, b0 + 2):
                    dst = vt_sb[p][b].rearrange(
                        "p (n d) -> p n d", n=16)[:, :, 32:64]
                    srcap = bass.AP(tensor=onesd.tensor, offset=0,
                                    ap=[[32, 128], [0, 16], [1, 32]])
                    nc.sync.dma_start(out=dst, in_=srcap.bitcast(F32R))

            def emit_cprev(p):
                cprev_sb[p] = sbA.tile([128, 512], F32, tag="cprev", name="cprev")
                nc.sync.dma_start(
                    out=cprev_sb[p],
                    in_=cin[2 * p:2 * p + 2].rearrange("s c q -> c s q"))

            # ---------------- attention iteration pieces ----------------
            def emit_score_group(p, b, s, ST, h, gg, first):
                """two K=32 row-partial score matmuls for head g=2h+gg."""
                kv = k_sb[p][b].rearrange("p (s c d) -> p s c d", s=2, c=2)
                qv = q_sb[p][b].rearrange("p (s q) -> p s q", s=2)
                g = 2 * h + gg
                if first and (h, gg) != (0, 0):
                    # first iteration has no previous-PV separators; use a
                    # full-row dummy into a slice the next score matmul
                    # overwrites (start=True clears it)
                    dsl = ST[0:32, 512:544] if gg == 1 else ST[0:32, 0:32]
                    nc.tensor.matmul(out=dsl, lhsT=W_["ones"],
                                     rhs=W_["ones"], start=True, stop=True,
                                     skip_group_check=True)
                for c in range(2):
                    nc.tensor.matmul(
                        out=ST[:, gg * 512 + c * 256:gg * 512 + c * 256 + 256],
                        lhsT=kv[32 * g:32 * g + 32, s, c, :],
                        rhs=qv[32 * g:32 * g + 32, s, :],
                        start=True, stop=True, skip_group_check=True,
                        tile_position=(32 * g, 0))

            def emit_pv_group(p, b, s, pT, AZ2, g):
                """full-row PV matmuls for head g into the two-head-stacked
                AZ2 [128, 512]: rows 64*(g%2)+[0:32]=a, +[32:64]=Z; columns
                256*(g//2). Serves as the row-group separator."""
                ro = 64 * (g % 2)
                co = 256 * (g // 2)
                for c in range(2):
                    sc = s * 2 + c
                    nc.tensor.matmul(
                        out=AZ2[ro:ro + 64, co:co + 256],
                        lhsT=vt_sb[p][b][:, sc * 256 + 64 * g:sc * 256 + 64 * g + 64],
                        rhs=pT[:, g * 512 + c * 256:g * 512 + c * 256 + 256],
                        start=(c == 0), stop=(c == 1), skip_group_check=True)

            def emit_norm(p, b, s, AZ2):
                """normalize straight out of PSUM, restack with 4 DMAs
                (2 HWDGE + 2 SWDGE)."""
                rz = sbB.tile([128, 512], F32, tag="rz", name="rz")
                nc.vector.reciprocal_approx_fast(out=rz, in_=AZ2)
                n2 = sbB.tile([64, 512], F32R, tag="n2", name="n2")
                nc.vector.tensor_mul(n2[0:32, :], AZ2[0:32, :], rz[32:64, :])
                nc.vector.tensor_mul(n2[32:64, :], AZ2[64:96, :], rz[96:128, :])
                if a_all[p] is None:
                    a_all[p] = sbA.tile([128, 2048], F32R, tag="aall", name="a_all")
                slot = b * 2 + s
                col = slot * 256
                for g in range(4):
                    src_ = n2[32 * (g % 2):32 * (g % 2) + 32,
                              256 * (g // 2):256 * (g // 2) + 256]
                    dst_ = a_all[p][32 * g:32 * g + 32, col:col + 256]
                    eng = nc.sync if g % 2 == 0 else nc.gpsimd
                    eng.dma_start(out=dst_, in_=src_)

            # ---------------- gates / state / output ----------------
            def emit_gate(p, gi):
                """full gate (pass 0): 4 token + 2 skip matmuls + act."""
                G = pwp.tile([128, 512], F32, tag="pw", name="G")
                av = a_all[p].rearrange("p (b s q) -> p b (s q)", b=4, s=2)
                for b in range(4):
                    nc.tensor.matmul(out=G, lhsT=W_["wtokT"][:, gi, b, :],
                                     rhs=av[:, b, :],
                                     start=(b == 0), stop=False)
                nc.tensor.matmul(out=G, lhsT=W_["wskipT"][:, gi, 0, :],
                                 rhs=xc_sb[p], start=False, stop=False)
                nc.tensor.matmul(out=G, lhsT=W_["wskipT"][:, gi, 1, :],
                                 rhs=hc_sb[p], start=False, stop=True)
                gate_sb[p][gi] = sbA.tile([128, 512], F32, tag=f"gate{gi}", name=f"gate{gi}")
                func = AF.Tanh if gi == 2 else AF.Sigmoid
                nc.scalar.activation(out=gate_sb[p][gi], in_=G, func=func,
                                     bias=W_["btok"][:, gi:gi + 1])

            def emit_gate_partial(p, gi):
                """pass-1 gate partial: skips + branches 3,1,2 -> SBUF."""
                G = pwp.tile([128, 512], F32, tag="pw", name="Gp")
                av = a_all[p].rearrange("p (b s q) -> p b (s q)", b=4, s=2)
                nc.tensor.matmul(out=G, lhsT=W_["wskipT"][:, gi, 0, :],
                                 rhs=xc_sb[p], start=True, stop=False)
                nc.tensor.matmul(out=G, lhsT=W_["wskipT"][:, gi, 1, :],
                                 rhs=hc_sb[p], start=False, stop=False)
                for j, b in enumerate((3, 1, 2)):
                    nc.tensor.matmul(out=G, lhsT=W_["wtokT"][:, gi, b, :],
                                     rhs=av[:, b, :],
                                     start=False, stop=(j == 2))
                gpart_sb[gi] = gpp.tile([128, 512], F32, tag=f"gp{gi}", name=f"gp{gi}")
                nc.vector.tensor_copy(gpart_sb[gi], G)

            def emit_gate_tail(p, gi):
                """pass-1 gate tail: branch-0 token matmuls (per s) + add
                partial + activation."""
                G = pwp.tile([128, 512], F32, tag="pw", name="Gt")
                av = a_all[p].rearrange("p (b s q) -> p b s q", b=4, s=2)
                for s in range(2):
                    nc.tensor.matmul(out=G[:, s * 256:(s + 1) * 256],
                                     lhsT=W_["wtokT"][:, gi, 0, :],
                                     rhs=av[:, 0, s, :],
                                     start=True, stop=True)
                nc.vector.tensor_add(gpart_sb[gi], G, gpart_sb[gi])
                gate_sb[p][gi] = sbA.tile([128, 512], F32, tag=f"gate{gi}", name=f"gate{gi}")
                func = AF.Tanh if gi == 2 else AF.Sigmoid
                nc.scalar.activation(out=gate_sb[p][gi], in_=gpart_sb[gi],
                                     func=func, bias=W_["btok"][:, gi:gi + 1])

            def emit_update_out(p):
                gi_, gf_, gg_, go_ = gate_sb[p]
                fc = sbA.tile([128, 512], F32, tag="fc", name="fc")
                nc.vector.tensor_mul(fc, gf_, cprev_sb[p])
                ig = sbA.tile([128, 512], F32, tag="ig", name="ig")
                nc.vector.tensor_mul(ig, gi_, gg_)
                cs = sbA.tile([128, 512], F32, tag="c", name="cs")
                nc.vector.tensor_add(cs, fc, ig)
                tcs = sbA.tile([128, 512], F32, tag="tc", name="tcs")
                nc.scalar.activation(out=tcs, in_=cs, func=AF.Tanh)
                hs = sbA.tile([128, 512], F32R, tag="h", name="hs")
                nc.vector.tensor_mul(hs, go_, tcs)
                OUT = pwp.tile([128, 512], F32, tag="pw", name="OUT")
                nc.tensor.matmul(out=OUT, lhsT=W_["woutT"], rhs=hs,
                                 start=True, stop=True)
                osb = sbA.tile([128, 512], F32, tag="out", name="osb")
                nc.vector.tensor_scalar_add(osb, OUT, W_["bout"][:, 0:1])
                nc.sync.dma_start(
                    out=yout[2 * p:2 * p + 2].rearrange("s c q -> c s q"),
                    in_=osb.rearrange("p (s q) -> p s q", s=2))

            # ---------------- emission schedule ----------------
            # weight loads, criticality-ordered (all on the SP HWDGE queue;
            # issue order == transfer order)
            W_["winT"] = wload("winT", winTd, [I, R])
            W_["b_in"] = wload("b_in", b_ind, [R, 1], F32)
            wct = wpool.tile([128, 2, 9, 128], F32R, tag="wconvT", name="wconvT")
            nc.sync.dma_start(out=wct[:, 1], in_=wconvTd.bitcast(F32R)[:, 1])
            W_["wconvT"] = wct

            # prologue: pass-0 pads + hc conv start as soon as inputs land
            xt_pad0, h_pad0 = emit_input_pads(0)
            emit_conv(0, 1, h_pad0)      # hc pass0

            W_["wqkT"] = wload("wqkT", wqkTd, [128, 2, 4, 128])
            emit_qk(0, 3)
            nc.sync.dma_start(out=wct[:, 0], in_=wconvTd.bitcast(F32R)[:, 0])
            W_["wvT"] = wload("wvT", wvTd, [128, 2, 256])
            W_["ones"] = wload("ones", onesd, [128, 32])
            emit_vt(0, 1)                # vT for b1, b3 (hc source)
            pads1 = [None]

            def load_late_weights():
                W_["wtokT"] = wload("wtokT", wtokTd, [128, 4, 4, 128])
                W_["wskipT"] = wload("wskipT", wskipTd, [128, 4, 2, 128])
                W_["woutT"] = wload("woutT", woutTd, [128, 128])
                W_["btok"] = wload("btok", btokd, [128, 4], F32)
                W_["bout"] = wload("bout", boutd, [128, 1], F32)

            def filler(i):
                if i == 0:
                    emit_conv(0, 0, xt_pad0)          # xc pass0
                elif i == 1:
                    emit_qk(0, 1)
                    emit_qk(0, 2)
                elif i == 2:
                    emit_qk(0, 0)
                    emit_vt(0, 0)
                    emit_cprev(0)
                elif i == 3:
                    pads1[0] = emit_input_pads(1)
                    load_late_weights()
                elif i == 4:
                    emit_conv(1, 1, pads1[0][1])      # hc pass1
                elif i == 5:
                    emit_conv(1, 0, pads1[0][0])      # xc pass1
                elif i == 6:
                    emit_qk(1, 3)
                    emit_vt(1, 1)
                elif i == 7:
                    emit_qk(1, 1)
                    emit_qk(1, 2)
                elif i == 8:
                    emit_qk(1, 0)
                    emit_vt(1, 0)
                    emit_cprev(1)
                elif i in (9, 10, 11, 12):
                    emit_gate(0, i - 9)
                elif i == 13:
                    emit_update_out(0)
                elif i == 14:
                    emit_gate_partial(1, 0)
                    emit_gate_partial(1, 1)
                elif i == 15:
                    emit_gate_partial(1, 2)
                    emit_gate_partial(1, 3)

            iters = [(p, b, s) for p in (0, 1) for b in BORDER for s in (0, 1)]
            prev = None
            for i, (p, b, s) in enumerate(iters):
                # interleave: score group g of iter i, then PV group g of
                # iter i-1 (full-row separator between row-partial groups)
                pT = sbB.tile([128, 2048], F32R, tag="pt", name="pT")
                AZ2p = None
                if prev is not None:
                    AZ2p = azp.tile([128, 512], F32, tag="az2", name="AZ2")
                for h in range(2):
                    ST = stp.tile([128, 1024], F32, tag="st", name="ST")
                    for gg in range(2):
                        g = 2 * h + gg
                        emit_score_group(p, b, s, ST, h, gg, prev is None)
                        if prev is not None:
                            pp, pb, ps, ppT = prev
                            emit_pv_group(pp, pb, ps, ppT, AZ2p, g)
                    nc.scalar.activation(out=pT[:, h * 1024:(h + 1) * 1024],
                                         in_=ST, func=AF.Exp)
                if prev is not None:
                    pp, pb, ps, ppT = prev
                    emit_norm(pp, pb, ps, AZ2p)
                prev = (p, b, s, pT)
                filler(i)
            # drain last iteration: PV + norm back-to-back (all full-row)
            pp, pb, ps, ppT = prev
            AZ2p = azp.tile([128, 512], F32, tag="az2", name="AZ2")
            for g in range(4):
                emit_pv_group(pp, pb, ps, ppT, AZ2p, g)
            emit_norm(pp, pb, ps, AZ2p)
            for gi in range(4):
                emit_gate_tail(1, gi)
            emit_update_out(1)

    nc.compile()
    return nc


def _prep_shared(inputs):
    f = np.float32
    c = np.ascontiguousarray
    W_cx, W_ch = np.asarray(inputs["W_cx"], f), np.asarray(inputs["W_ch"], f)
    W_q, W_k, W_v = (np.asarray(inputs[k], f) for k in ("W_q", "W_k", "W_v"))
    W_tok, W_skip = np.asarray(inputs["W_tok"], f), np.asarray(inputs["W_skip"], f)
    shared = {
        "winT": c(np.asarray(inputs["W_in"], f).T),
        "b_in": c(np.asarray(inputs["b_in"], f).reshape(R, 1)),
        # [i, src, tap, o]
        "wconvT": c(np.stack([W_cx.transpose(1, 2, 3, 0).reshape(128, 9, 128),
                              W_ch.transpose(1, 2, 3, 0).reshape(128, 9, 128)],
                             axis=1)),
        # [c, (q|k), b, a]
        "wqkT": c(np.stack([W_q.transpose(2, 0, 1), W_k.transpose(2, 0, 1)],
                           axis=1)),
        # [c, srcpair, a-pair]: xc feeds branches (0,2), hc feeds (1,3)
        "wvT": c(np.stack([
            np.concatenate([W_v[0].T, W_v[2].T], axis=1),
            np.concatenate([W_v[1].T, W_v[3].T], axis=1)], axis=1)),
        "onesd": np.ones((128, 32), f),
        # [a, gate, branch, r]
        "wtokT": c(W_tok.transpose(3, 0, 1, 2)),
        "btok": c(np.asarray(inputs["b_tok"], f).T),
        # [c, gate, src, r]
        "wskipT": c(W_skip.transpose(3, 0, 1, 2)),
        "woutT": c(np.asarray(inputs["W_out"], f).T),
        "bout": c(np.asarray(inputs["b_out"], f).reshape(R, 1)),
    }
    return shared


def kernel(**inputs):
    from concourse.bass_utils import run_bass_kernel_spmd
    if "nc" not in _CACHE:
        _CACHE["nc"] = _build_program()
    nc = _CACHE["nc"]
    f = np.float32
    x = np.asarray(inputs["x"], f).reshape(N, I, HW)
    hp = np.asarray(inputs["h_prev"], f).reshape(N, R, HW)
    cp = np.asarray(inputs["c_prev"], f).reshape(N, R, HW)
    shared = _prep_shared(inputs)
    in_maps = []
    for ci in range(NCORES):
        sl = slice(S * ci, S * ci + S)
        m = dict(shared)
        m["xin"] = np.ascontiguousarray(x[sl])
        m["hin"] = np.ascontiguousarray(hp[sl])
        m["cin"] = np.ascontiguousarray(cp[sl])
        in_maps.append(m)
    res = run_bass_kernel_spmd(nc, in_maps, core_ids=list(range(NCORES)))
    y = np.concatenate([r["yout"].reshape(S, R, H, W) for r in res.results],
                       axis=0)
    return y.astype(np.float32)
